# revision 1
# baseline (speedup 1.0000x reference)
"""Trainium2 Bass kernel for nn_Blur: depthwise 4x4 FIR conv, pad=2.

out[b,c,h',w'] = sum_{i,j} wf[i,j] * xpad[b,c,h'+i,w'+j],  wf = flip(kernel)
x: [8,256,256,256] f32, kernel: [4,4] f32 -> out: [8,256,257,257] f32

Strategy: pure data parallel over batch (8 cores, 1 batch elem each).
Per core, the full 2D conv runs on the TensorEngine as 4 banded-matrix
matmuls (one per kernel column j) accumulating in PSUM:
    psum[h', w'] += sum_h B_j[h,h'] * xpad_w[h, w'+j]
with B_j[h,h'] = wf[h-h'+2, j] built on the host from the runtime kernel.
float32r dtype gives full PE rate (1 cyc/row at N>=256, even N required);
rel err ~2e-4, far inside the 2e-2 gate.

DMA layout: 8 channels batched per transfer (~1 MB per dma_start) to
amortize the ~0.8-2us per-DMA completion latency; input loads issue on
the SP HWDGE ring (nc.sync), output stores on the ACT ring (nc.scalar)
so the two FIFO rings overlap.
"""

import numpy as np

_C, _H, _W = 256, 256, 256
_HO, _WO = 257, 257
_NCORES = 8
# (hp0, Mv, hlo, Kv): output rows [hp0, hp0+Mv), contraction rows [hlo, hlo+Kv)
_TILES = [(0, 125, 0, 126), (125, 125, 123, 128), (250, 7, 248, 8)]
_NW = 262  # padded width in SBUF: 2 zero | 256 data | 4 zero
_NMM = 258  # matmul free dim (257 outputs + 1 garbage col), must be even
_OPAD = 264  # padded output row pitch in DRAM (1056B, 32B-aligned)
_CB = 8  # channels per DMA batch / psum rotation


def _build_bands(kern):
    wf = np.ascontiguousarray(np.asarray(kern, np.float32)[::-1, ::-1])
    bands = np.zeros((128, 3, 4, 125), np.float32)
    for v, (hp0, Mv, hlo, Kv) in enumerate(_TILES):
        for j in range(4):
            for hr in range(Kv):
                h = hlo + hr
                for mr in range(Mv):
                    i = h - (hp0 + mr) + 2
                    if 0 <= i < 4:
                        bands[hr, v, j, mr] = wf[i, j]
    return bands


_NC_CACHE = {}


def _build_nc():
    if "nc" in _NC_CACHE:
        return _NC_CACHE["nc"]
    import concourse.bacc as bacc
    import concourse.mybir as mybir
    import concourse.tile as tile

    nc = bacc.Bacc()
    x_d = nc.declare_dram_parameter("x", [_C, _H, _W], mybir.dt.float32r, isOutput=False)
    b_d = nc.declare_dram_parameter(
        "bands", [128, 3, 4, 125], mybir.dt.float32r, isOutput=False
    )
    o_d = nc.declare_dram_parameter("out", [_C, _HO, _OPAD], mybir.dt.float32, isOutput=True)
    z_d = nc.declare_dram_parameter("zpad", [128, _CB, 4], mybir.dt.float32r, isOutput=False)

    NBX = 4  # x-tile ring depth (each tile holds a whole channel-group's rows)
    NBO = 4  # out-tile ring depth
    NBP = 8  # psum banks: one per channel within a group
    with tile.TileContext(nc) as tc:
        with (
            tc.tile_pool(name="sb", bufs=1) as pool,
            tc.tile_pool(name="ps", bufs=1, space="PSUM") as pp,
        ):
            band_sb = pool.tile([128, 3, 4, 125], mybir.dt.float32r, tag="bands")
            nc.sync.dma_start(out=band_sb[:], in_=b_d[:])

            xts = []
            for i in range(NBX):
                t = pool.tile(
                    [128, _CB, _NW], mybir.dt.float32r, tag=f"xt{i}", name=f"xt{i}"
                )
                nc.sync.dma_start(out=t[:, :, 0:2], in_=z_d[:, :, 0:2])
                nc.sync.dma_start(out=t[:, :, 258:_NW], in_=z_d[:, :, 0:4])
                xts.append(t)
            oss = [
                pool.tile(
                    [128, _CB, _OPAD], mybir.dt.float32, tag=f"os{i}", name=f"os{i}"
                )
                for i in range(NBO)
            ]
            pss = [
                pp.tile([128, _NMM], mybir.dt.float32, tag=f"ps{i}", name=f"ps{i}")
                for i in range(NBP)
            ]

            it = 0
            for c0 in range(0, _C, _CB):
                for v, (hp0, Mv, hlo, Kv) in enumerate(_TILES):
                    xt = xts[it % NBX]
                    osb = oss[it % NBO]
                    nc.sync.dma_start(
                        out=xt[0:Kv, :, 2:258],
                        in_=x_d[c0 : c0 + _CB, hlo : hlo + Kv, :].rearrange(
                            "c h w -> h c w"
                        ),
                    )
                    for cc in range(_CB):
                        ps = pss[cc]
                        for j in range(4):
                            nc.tensor.matmul(
                                ps[0:Mv, 0:_NMM],
                                band_sb[0:Kv, v, j, 0:Mv],
                                xt[0:Kv, cc, j : j + _NMM],
                                start=(j == 0),
                                stop=(j == 3),
                            )
                        if cc % 2 == 0:
                            nc.vector.tensor_copy(
                                osb[0:Mv, cc, 0:_WO], ps[0:Mv, 0:_WO]
                            )
                        else:
                            nc.scalar.copy(osb[0:Mv, cc, 0:_WO], ps[0:Mv, 0:_WO])
                    nc.gpsimd.dma_start(
                        out=o_d[c0 : c0 + _CB, hp0 : hp0 + Mv, :].rearrange(
                            "c h w -> h c w"
                        ),
                        in_=osb[0:Mv, :, 0:_OPAD],
                    )
                    it += 1
    nc.finalize()
    _NC_CACHE["nc"] = nc
    return nc


def _run(x, kern, trace=False):
    from concourse.bass_utils import run_bass_kernel_spmd

    x = np.asarray(x, dtype=np.float32)
    bands = _build_bands(kern)
    nc = _build_nc()
    zpad = np.zeros((128, _CB, 4), np.float32)
    in_maps = [
        {"x": np.ascontiguousarray(x[b]), "bands": bands, "zpad": zpad}
        for b in range(_NCORES)
    ]
    res = run_bass_kernel_spmd(nc, in_maps, list(range(_NCORES)), trace=trace)
    out = np.stack(
        [np.asarray(res.results[i]["out"])[:, :, : _WO] for i in range(_NCORES)],
        axis=0,
    ).astype(np.float32)
    return out, res


def kernel(x, kernel):
    out, _ = _run(x, kernel, trace=False)
    return out



# revision 2
# speedup vs baseline: 1.0721x; 1.0721x over previous
"""Trainium2 Bass kernel for nn_Blur: depthwise 4x4 FIR conv, pad=2. v2.

out[b,c,h',w'] = sum_{i,j} wf[i,j] * xpad[b,c,h'+i,w'+j],  wf = flip(kernel)
x: [8,256,256,256] f32, kernel: [4,4] f32 -> out: [8,256,257,257] f32

v2 changes vs v1 (675us):
- fp16 end-to-end on device: host converts x -> fp16 [H,C,W] layout, device
  stores fp16 [HO,C,WO]; host converts back. Halves HBM traffic (139->69MB
  per core) and gives 4KB contiguous DMA segments per partition line
  (vs 1KB/1056B), fixing per-packet overhead.
- Output rows 0..249 via 2 TensorE tiles (Mv=125 each); the ragged 7-row
  tail (250..256) moves to DVE FMAs (scalar_tensor_tensor, runtime scalars
  from a broadcast wf tile), saving 1/3 of the 258-col matmul streams.
- j-outer / cc-inner matmul order: 8 consecutive matmuls share the same
  stationary band matrix (LDWEIGHTS amortization).
- psum->sbuf convert copies split across vector/scalar/gpsimd.
"""

import numpy as np

_C, _H, _W = 256, 256, 256
_HO, _WO = 257, 257
_NCORES = 8
# (hp0, Mv, hlo, Kv): output rows [hp0, hp0+Mv), contraction image rows
# [hlo, hlo+Kv).  Rows 250..256 are computed on the DVE (strip path).
_TILES = [(0, 125, 0, 126), (125, 125, 123, 128)]
_NW = 262  # padded width in SBUF: 2 zero | 256 data | 4 zero
_NMM = 258  # matmul free dim (257 outputs + 1 garbage col)
_CB = 8  # channels per DMA batch / psum rotation
_NTAIL = 7  # output rows 250..256 on DVE
_FMA_START = 8  # first main-loop iteration that also emits a tail FMA


def _build_bands(kern):
    wf = np.ascontiguousarray(np.asarray(kern, np.float32)[::-1, ::-1])
    bands = np.zeros((128, 2, 4, 125), np.float32)
    for v, (hp0, Mv, hlo, Kv) in enumerate(_TILES):
        for j in range(4):
            for hr in range(Kv):
                h = hlo + hr
                for mr in range(Mv):
                    i = h - (hp0 + mr) + 2
                    if 0 <= i < 4:
                        bands[hr, v, j, mr] = wf[i, j]
    return bands.astype(np.float16), wf


_NC_CACHE = {}


def _build_nc():
    if "nc" in _NC_CACHE:
        return _NC_CACHE["nc"]
    import concourse.bacc as bacc
    import concourse.mybir as mybir
    import concourse.tile as tile

    f16 = mybir.dt.float16
    f32 = mybir.dt.float32

    nc = bacc.Bacc()
    x_d = nc.declare_dram_parameter("x", [_H, _C, _W], f16, isOutput=False)
    b_d = nc.declare_dram_parameter("bands", [128, 2, 4, 125], f16, isOutput=False)
    s_d = nc.declare_dram_parameter("strip", [128, 2, 10, 260], f32, isOutput=False)
    w_d = nc.declare_dram_parameter("wfbc", [128, 16], f32, isOutput=False)
    z_d = nc.declare_dram_parameter("zpad", [128, _CB, 4], f16, isOutput=False)
    o_d = nc.declare_dram_parameter("out", [_HO, _C, _WO], f16, isOutput=True)

    NBX = 4  # x-tile ring depth
    NBO = 4  # out-tile ring depth
    NBP = 8  # psum banks: one per channel within a group
    TAPS = [(i, j) for i in range(4) for j in range(4)]
    with tile.TileContext(nc) as tc:
        with (
            tc.tile_pool(name="sb", bufs=1) as pool,
            tc.tile_pool(name="ps", bufs=1, space="PSUM") as pp,
        ):
            band_sb = pool.tile([128, 2, 4, 125], f16, tag="bands")
            nc.sync.dma_start(out=band_sb[:], in_=b_d[:])
            strip_sb = pool.tile([128, 2, 10, 260], f32, tag="strip")
            nc.sync.dma_start(out=strip_sb[:], in_=s_d[:])
            wf_sb = pool.tile([128, 16], f32, tag="wf")
            nc.sync.dma_start(out=wf_sb[:], in_=w_d[:])

            xts = []
            for i in range(NBX):
                t = pool.tile([128, _CB, _NW], f16, tag=f"xt{i}", name=f"xt{i}")
                nc.sync.dma_start(out=t[:, :, 0:2], in_=z_d[:, :, 0:2])
                nc.sync.dma_start(out=t[:, :, 258:_NW], in_=z_d[:, :, 0:4])
                xts.append(t)
            oss = [
                pool.tile([128, _CB, _WO], f16, tag=f"os{i}", name=f"os{i}")
                for i in range(NBO)
            ]
            # full 2KB bank per tile so accumulation groups never share a bank
            pss = [
                pp.tile([128, 512], f32, tag=f"ps{i}", name=f"ps{i}")
                for i in range(NBP)
            ]
            accA = pool.tile([128, 2, _NTAIL, _WO], f32, tag="accA")
            accB = pool.tile([128, 2, _NTAIL, _WO], f32, tag="accB")
            acc16 = pool.tile([128, 2, _NTAIL, _WO], f16, tag="acc16")

            def emit_tail_fma(k):
                g, t = k % 2, k // 2
                i, j = TAPS[t]
                src = strip_sb[:, g, i : i + _NTAIL, j : j + _WO]
                sc = wf_sb[:, t : t + 1]
                if t == 0:
                    nc.vector.tensor_scalar_mul(accA[:, g], src, sc)
                else:
                    dst, prev = (accA, accB) if t % 2 == 0 else (accB, accA)
                    nc.vector.scalar_tensor_tensor(
                        out=dst[:, g],
                        in0=src,
                        scalar=sc,
                        in1=prev[:, g],
                        op0=mybir.AluOpType.mult,
                        op1=mybir.AluOpType.add,
                    )
                if t == 15:
                    nc.vector.tensor_copy(acc16[:, g], accB[:, g])
                    nc.gpsimd.dma_start(
                        out=o_d[250:_HO, g * 128 : (g + 1) * 128, :].rearrange(
                            "h c w -> c h w"
                        ),
                        in_=acc16[:, g],
                    )

            it = 0
            for c0 in range(0, _C, _CB):
                for v, (hp0, Mv, hlo, Kv) in enumerate(_TILES):
                    xt = xts[it % NBX]
                    osb = oss[it % NBO]
                    nc.sync.dma_start(
                        out=xt[0:Kv, :, 2:258], in_=x_d[hlo : hlo + Kv, c0 : c0 + _CB, :]
                    )
                    for j in range(4):
                        for cc in range(_CB):
                            nc.tensor.matmul(
                                pss[cc][0:Mv, 0:_NMM],
                                band_sb[0:Kv, v, j, 0:Mv],
                                xt[0:Kv, cc, j : j + _NMM],
                                start=(j == 0),
                                stop=(j == 3),
                            )
                    for cc in range(_CB):
                        ps = pss[cc]
                        if cc < 3:
                            nc.vector.tensor_copy(osb[0:Mv, cc, 0:_WO], ps[0:Mv, 0:_WO])
                        else:
                            nc.scalar.copy(osb[0:Mv, cc, 0:_WO], ps[0:Mv, 0:_WO])
                    nc.gpsimd.dma_start(
                        out=o_d[hp0 : hp0 + Mv, c0 : c0 + _CB, :], in_=osb[0:Mv, :, :]
                    )
                    k = it - _FMA_START
                    if 0 <= k < 32:
                        emit_tail_fma(k)
                    it += 1
    nc.finalize()
    _NC_CACHE["nc"] = nc
    return nc


def _prep_core_inputs(x, bands16, wfbc, b):
    xb = x[b]  # [C, H, W] f32
    xT = np.ascontiguousarray(xb.transpose(1, 0, 2).astype(np.float16, order="C"))
    strip = np.zeros((128, 2, 10, 260), np.float32)
    bot = xb[:, 248:256, :]  # [256, 8, 256]
    strip[:, 0, 0:8, 2:258] = bot[0:128]
    strip[:, 1, 0:8, 2:258] = bot[128:256]
    zpad = np.zeros((128, _CB, 4), np.float16)
    return {"x": xT, "bands": bands16, "strip": strip, "wfbc": wfbc, "zpad": zpad}


def _run(x, kern, trace=False):
    from concourse.bass_utils import run_bass_kernel_spmd

    x = np.asarray(x, dtype=np.float32)
    bands16, wf = _build_bands(kern)
    wfbc = np.ascontiguousarray(
        np.broadcast_to(wf.reshape(1, 16), (128, 16)).astype(np.float32)
    )
    nc = _build_nc()
    in_maps = [_prep_core_inputs(x, bands16, wfbc, b) for b in range(_NCORES)]
    res = run_bass_kernel_spmd(nc, in_maps, list(range(_NCORES)), trace=trace)
    out = np.stack(
        [
            np.asarray(res.results[i]["out"]).transpose(1, 0, 2).astype(np.float32)
            for i in range(_NCORES)
        ],
        axis=0,
    )
    return out, res


def kernel(x, kernel):
    out, _ = _run(x, kernel, trace=False)
    return out


# revision 4
# speedup vs baseline: 1.1678x; 1.0893x over previous
"""Trainium2 Bass kernel for nn_Blur: depthwise 4x4 FIR conv, pad=2. v6.

v6 = v3 (302us) + CB=16: 16 channels per DMA batch doubles the bytes per
DMA descriptor row (8384B loads / 8224B stores), halving descriptor and
issue counts to unclamp the per-ring dispatch rate. Matmuls run in two
8-channel sub-batches per tile (8 psum banks). Tail FMAs emit one per
tile iteration (32 tiles, 32 FMAs). Copies split 6 vector / 10 scalar.
Stores stay on the gpsimd SWDGE ring (v4/v5 showed scalar-ring stores +
overloaded vector regress).
"""

import numpy as np

_C, _H, _W = 256, 256, 256
_HO, _WO = 257, 257
_NCORES = 8
_TILES = [(0, 125, 0, 126), (125, 125, 123, 128)]
_XW = 262  # per-channel padded width: 2 zero | 256 data | 4 zero
_NMM = 258
_CB = 16  # channels per DMA batch; 2 psum sub-batches of 8
_NTAIL = 7
_FMA_START = 0  # 32 iterations, 32 tail FMAs: one per iteration


def _build_bands(kern):
    wf = np.ascontiguousarray(np.asarray(kern, np.float32)[::-1, ::-1])
    bands = np.zeros((128, 2, 4, 125), np.float32)
    for v, (hp0, Mv, hlo, Kv) in enumerate(_TILES):
        for j in range(4):
            for hr in range(Kv):
                h = hlo + hr
                for mr in range(Mv):
                    i = h - (hp0 + mr) + 2
                    if 0 <= i < 4:
                        bands[hr, v, j, mr] = wf[i, j]
    return bands.astype(np.float16), wf


_NC_CACHE = {}


def _build_nc():
    if "nc" in _NC_CACHE:
        return _NC_CACHE["nc"]
    import concourse.bacc as bacc
    import concourse.mybir as mybir
    import concourse.tile as tile

    f16 = mybir.dt.float16
    f32 = mybir.dt.float32

    nc = bacc.Bacc()
    x_d = nc.declare_dram_parameter("x", [_H, _C * _XW], f16, isOutput=False)
    b_d = nc.declare_dram_parameter("bands", [128, 2, 4, 125], f16, isOutput=False)
    s_d = nc.declare_dram_parameter("strip", [128, 2, 10, 260], f32, isOutput=False)
    w_d = nc.declare_dram_parameter("wfbc", [128, 16], f32, isOutput=False)
    o_d = nc.declare_dram_parameter("out", [_HO, _C * _WO], f16, isOutput=True)
    t_d = nc.declare_dram_parameter("tail", [128, 2, _NTAIL, _WO], f16, isOutput=True)

    NBX = 4
    NBO = 4
    NBP = 8
    XTW = _CB * _XW  # 4192
    OSW = _CB * _WO  # 4112
    TAPS = [(i, j) for i in range(4) for j in range(4)]
    with tile.TileContext(nc) as tc:
        with (
            tc.tile_pool(name="sb", bufs=1) as pool,
            tc.tile_pool(name="ps", bufs=1, space="PSUM") as pp,
        ):
            band_sb = pool.tile([128, 2, 4, 125], f16, tag="bands")
            nc.sync.dma_start(out=band_sb[:], in_=b_d[:])
            strip_sb = pool.tile([128, 2, 10, 260], f32, tag="strip")
            nc.sync.dma_start(out=strip_sb[:], in_=s_d[:])
            wf_sb = pool.tile([128, 16], f32, tag="wf")
            nc.sync.dma_start(out=wf_sb[:], in_=w_d[:])

            xts = [
                pool.tile([128, XTW], f16, tag=f"xt{i}", name=f"xt{i}")
                for i in range(NBX)
            ]
            oss = [
                pool.tile([128, OSW], f16, tag=f"os{i}", name=f"os{i}")
                for i in range(NBO)
            ]
            pss = [
                pp.tile([128, 512], f32, tag=f"ps{i}", name=f"ps{i}")
                for i in range(NBP)
            ]
            accA = pool.tile([128, 2, _NTAIL, _WO], f32, tag="accA")
            accB = pool.tile([128, 2, _NTAIL, _WO], f32, tag="accB")
            acc16 = pool.tile([128, 2, _NTAIL, _WO], f16, tag="acc16")

            def emit_tail_fma(k):
                g, t = k % 2, k // 2
                i, j = TAPS[t]
                src = strip_sb[:, g, i : i + _NTAIL, j : j + _WO]
                sc = wf_sb[:, t : t + 1]
                if t == 0:
                    nc.vector.tensor_scalar_mul(accA[:, g], src, sc)
                else:
                    dst, prev = (accA, accB) if t % 2 == 0 else (accB, accA)
                    nc.vector.scalar_tensor_tensor(
                        out=dst[:, g],
                        in0=src,
                        scalar=sc,
                        in1=prev[:, g],
                        op0=mybir.AluOpType.mult,
                        op1=mybir.AluOpType.add,
                    )
                if t == 15:
                    nc.vector.tensor_copy(acc16[:, g], accB[:, g])
                    if g == 1:
                        nc.gpsimd.dma_start(out=t_d[:], in_=acc16[:])

            it = 0
            for c0 in range(0, _C, _CB):
                for v, (hp0, Mv, hlo, Kv) in enumerate(_TILES):
                    xt = xts[it % NBX]
                    osb = oss[it % NBO]
                    k = it - _FMA_START
                    if 0 <= k < 32:
                        emit_tail_fma(k)
                    nc.sync.dma_start(
                        out=xt[0:Kv, 0:XTW],
                        in_=x_d[hlo : hlo + Kv, c0 * _XW : c0 * _XW + XTW],
                    )
                    for half in range(2):
                        for j in range(4):
                            for c8 in range(8):
                                cc = half * 8 + c8
                                nc.tensor.matmul(
                                    pss[c8][0:Mv, 0:_NMM],
                                    band_sb[0:Kv, v, j, 0:Mv],
                                    xt[0:Kv, cc * _XW + j : cc * _XW + j + _NMM],
                                    start=(j == 0),
                                    stop=(j == 3),
                                )
                        for c8 in range(8):
                            cc = half * 8 + c8
                            ps = pss[c8]
                            if c8 < 3:
                                nc.vector.tensor_copy(
                                    osb[0:Mv, cc * _WO : cc * _WO + _WO],
                                    ps[0:Mv, 0:_WO],
                                )
                            else:
                                nc.scalar.copy(
                                    osb[0:Mv, cc * _WO : cc * _WO + _WO],
                                    ps[0:Mv, 0:_WO],
                                )
                    nc.gpsimd.dma_start(
                        out=o_d[hp0 : hp0 + Mv, c0 * _WO : c0 * _WO + OSW],
                        in_=osb[0:Mv, 0:OSW],
                    )
                    it += 1
    nc.finalize()
    _NC_CACHE["nc"] = nc
    return nc


def _prep_core_inputs(x, bands16, wfbc, b):
    xb = x[b]  # [C, H, W] f32
    xT = np.zeros((_H, _C, _XW), np.float16)
    xT[:, :, 2:258] = xb.transpose(1, 0, 2).astype(np.float16, order="C")
    strip = np.zeros((128, 2, 10, 260), np.float32)
    bot = xb[:, 248:256, :]
    strip[:, 0, 0:8, 2:258] = bot[0:128]
    strip[:, 1, 0:8, 2:258] = bot[128:256]
    return {
        "x": xT.reshape(_H, _C * _XW),
        "bands": bands16,
        "strip": strip,
        "wfbc": wfbc,
    }


def _run(x, kern, trace=False):
    from concourse.bass_utils import run_bass_kernel_spmd

    x = np.asarray(x, dtype=np.float32)
    bands16, wf = _build_bands(kern)
    wfbc = np.ascontiguousarray(
        np.broadcast_to(wf.reshape(1, 16), (128, 16)).astype(np.float32)
    )
    nc = _build_nc()
    in_maps = [_prep_core_inputs(x, bands16, wfbc, b) for b in range(_NCORES)]
    res = run_bass_kernel_spmd(nc, in_maps, list(range(_NCORES)), trace=trace)
    outs = []
    for i in range(_NCORES):
        o = (
            np.asarray(res.results[i]["out"])
            .reshape(_HO, _C, _WO)
            .transpose(1, 0, 2)
            .astype(np.float32)
        )
        tail = np.asarray(res.results[i]["tail"]).astype(np.float32)
        o[0:128, 250:_HO, :] = tail[:, 0]
        o[128:256, 250:_HO, :] = tail[:, 1]
        outs.append(o)
    return np.stack(outs, axis=0), res


def kernel(x, kernel):
    out, _ = _run(x, kernel, trace=False)
    return out
